# revision 3
# baseline (speedup 1.0000x reference)
"""LocalMHA (windowed attention, window=128, look_backward=1, RoPE) on 8 TRN2 cores.

Sharding: sequence-parallel. Core c handles batch c//2, sequence half c%2
(2048 query tokens + a 128-token look-backward halo). No collectives: the
halo tokens are re-projected locally (halo rows of x are shipped in the
core's input shard; zeros at a true sequence start, where the mask kills
the backward keys anyway).

Per-core pipeline (all matmuls in fp32r — full-rate on PE, ~1.6e-4 rel err):
  A. x tiles -> PE transpose -> xT [dim, tok] chunks in SBUF
  B. QKV projection. q^T and k^T produced head-transposed ([h*dh, tok]);
     RoPE applied on the fly (k twice: once with its own-window phase, once
     with next-window backward phase) and staged to DRAM. v produced in
     natural [tok, h*dh] layout and staged.
  C. Per window w, per head: scores = q'^T.T @ k2 (free dim 256), banded
     causal mask add (DVE), exp+rowsum (ACT, fused accum), normalize (DVE),
     PE-transpose p, attn_out^T = v.T-chunks @ p^T accumulated; written to a
     resident SBUF accumulator aT [h*dh, 2048].
  D. out = aT.T @ W_out, streamed to DRAM.
"""

import numpy as np
from contextlib import ExitStack

import concourse.bacc as bacc
import concourse.tile as tile
import concourse.mybir as mybir
from concourse.bass_utils import run_bass_kernel_spmd
from concourse.masks import make_identity

# Problem shape (hardcoded per contract)
B, N, D = 4, 4096, 1024
H, DH, WS = 16, 64, 128
THETA = 10000.0
N3 = 3 * H * DH            # 3072
NCORES = 8
HALF = N // 2              # 2048 query tokens per core
NT = HALF + WS             # 2176 tokens incl halo window
NWIN = HALF // WS          # 16 query windows
SCALE = DH ** -0.5
NEG = -1.0e9

F32 = mybir.dt.float32
F32R = mybir.dt.float32r
ADD = mybir.AluOpType.add
MUL = mybir.AluOpType.mult
EXP = mybir.ActivationFunctionType.Exp

# token chunks for phases A/B (start, len); 128-aligned, len<=512
CHUNKS = [(0, 512), (512, 512), (1024, 512), (1536, 512), (2048, 128)]


def _rope(nc, tmpp, craw, dst, L, rp, ci, si):
    """dst[:, :L] = craw*cos + swap64(craw)*sin_signed (f32r out).

    craw rows are two 64-row head blocks; rotate_half exchanges the 32-row
    halves within each block. The sin tile is shipped indexed by SOURCE row
    (sign flipped host-side: sin(theta_r) is invariant under r^32, only the
    sign changes), so both tensor_tensor inputs share a base partition —
    only the output is partition-shifted, which the ISA allows.
    """
    t1 = tmpp.tile([128, 512], F32, tag="t1")
    nc.vector.tensor_tensor(t1[:, :L], craw[:, :L], rp[:, ci, :L], MUL)
    t2 = tmpp.tile([128, 512], F32, tag="t2")
    for hb in range(2):
        o = hb * 64
        nc.vector.tensor_tensor(t2[o:o + 32, :L], craw[o + 32:o + 64, :L],
                                rp[o + 32:o + 64, si, :L], MUL)
        nc.vector.tensor_tensor(t2[o + 32:o + 64, :L], craw[o:o + 32, :L],
                                rp[o:o + 32, si, :L], MUL)
    nc.vector.tensor_tensor(dst[:, :L], t1[:, :L], t2[:, :L], ADD)


def _build():
    nc = bacc.Bacc("TRN2", target_bir_lowering=False, debug=False,
                   enable_asserts=False, num_devices=NCORES)

    xs = nc.dram_tensor("xs", [NT, D], F32R, kind="ExternalInput").ap()
    wqkv = nc.dram_tensor("wqkv", [D, N3], F32R, kind="ExternalInput").ap()
    wout = nc.dram_tensor("wout", [D, D], F32R, kind="ExternalInput").ap()
    # 0:qcos 1:qsin 2:kcos_cur 3:ksin_cur 4:kcos_prev 5:ksin_prev
    ropes = nc.dram_tensor("ropes", [6, 128, 512], F32, kind="ExternalInput").ap()
    masks = nc.dram_tensor("masks", [2, 128, 256], F32, kind="ExternalInput").ap()
    out = nc.dram_tensor("out", [HALF, D], F32, kind="ExternalOutput").ap()

    # internal DRAM staging
    qrope = nc.dram_tensor("qrope", [D, HALF], F32R).ap()
    k2 = nc.dram_tensor("k2", [D, NWIN, 2 * WS], F32R).ap()
    vstage = nc.dram_tensor("vstage", [NT, D], F32R).ap()

    with tile.TileContext(nc) as tc:
        with ExitStack() as top:
            constp = top.enter_context(tc.tile_pool(name="const", bufs=1))
            identf = constp.tile([128, 128], F32, tag="idf")
            make_identity(nc, identf[:])
            ident = constp.tile([128, 128], F32R, tag="idr")
            nc.vector.tensor_copy(ident[:], identf[:])
            rp = constp.tile([128, 6, 512], F32, tag="ropes")
            nc.sync.dma_start(rp[:], ropes.rearrange("r p m -> p r m"))
            mk = constp.tile([128, 2, 256], F32, tag="masks")
            nc.sync.dma_start(mk[:], masks.rearrange("r p m -> p r m"))

            # ---------------- Phase A+B: transpose + QKV + RoPE ----------------
            with ExitStack() as ab:
                wp = ab.enter_context(tc.tile_pool(name="wq", bufs=1))
                w_sb = wp.tile([128, 8, N3], F32R, tag="w")
                nc.sync.dma_start(w_sb[:],
                                  wqkv.rearrange("(c p) n -> p c n", p=128))
                xp = ab.enter_context(tc.tile_pool(name="xst", bufs=2))
                xtp = ab.enter_context(tc.tile_pool(name="xT", bufs=2))
                crp = ab.enter_context(tc.tile_pool(name="craw", bufs=3))
                tmpp = ab.enter_context(tc.tile_pool(name="tmp", bufs=2))
                rop = ab.enter_context(tc.tile_pool(name="ro", bufs=4))
                vp = ab.enter_context(tc.tile_pool(name="vsb", bufs=3))
                tps = ab.enter_context(tc.tile_pool(name="tps", bufs=3, space="PSUM"))
                mps = ab.enter_context(tc.tile_pool(name="mps", bufs=4, space="PSUM"))

                for (s, L) in CHUNKS:
                    nmt = L // 128
                    xT = xtp.tile([128, 8, 512], F32R, tag="xT")
                    for mt in range(nmt):
                        xst = xp.tile([128, D], F32R, tag="x")
                        nc.sync.dma_start(xst[:], xs[s + mt * 128: s + (mt + 1) * 128, :])
                        for kc in range(8):
                            tp = tps.tile([128, 128], F32R, tag="tp")
                            nc.tensor.transpose(tp[:], xst[:, kc * 128:(kc + 1) * 128],
                                                ident[:])
                            nc.vector.tensor_copy(xT[:, kc, mt * 128:(mt + 1) * 128], tp[:])

                    # q^T (+rope) for query tokens of this chunk
                    qs = 128 if s == 0 else 0
                    qL = L - qs
                    if qL > 0:
                        for nch in range(8):
                            qp = mps.tile([128, 512], F32, tag="mm")
                            for kc in range(8):
                                nc.tensor.matmul(qp[:, :qL],
                                                 w_sb[:, kc, nch * 128:(nch + 1) * 128],
                                                 xT[:, kc, qs:qs + qL],
                                                 start=(kc == 0), stop=(kc == 7))
                            craw = crp.tile([128, 512], F32, tag="craw")
                            nc.scalar.copy(craw[:, :qL], qp[:, :qL])
                            qf = rop.tile([128, 512], F32R, tag="ro")
                            _rope(nc, tmpp, craw, qf, qL, rp, 0, 1)
                            q0 = s + qs - 128
                            nc.sync.dma_start(
                                qrope[nch * 128:(nch + 1) * 128, q0:q0 + qL],
                                qf[:, :qL])

                    # k^T with both rope phases
                    for nch in range(8):
                        kp = mps.tile([128, 512], F32, tag="mm")
                        for kc in range(8):
                            nc.tensor.matmul(kp[:, :L],
                                             w_sb[:, kc, 1024 + nch * 128: 1024 + (nch + 1) * 128],
                                             xT[:, kc, 0:L],
                                             start=(kc == 0), stop=(kc == 7))
                        craw = crp.tile([128, 512], F32, tag="craw")
                        nc.scalar.copy(craw[:, :L], kp[:, :L])
                        # current-window phase: token t -> K2[w(t)-1? no: w_cur, 128:256]
                        cs = 128 if s == 0 else 0     # halo window has no cur slot
                        if L - cs > 0:
                            kcur = rop.tile([128, 512], F32R, tag="ro")
                            _rope(nc, tmpp, craw, kcur, L, rp, 2, 3)
                            w0 = (s + cs) // 128 - 1
                            nw = (L - cs) // 128
                            nc.sync.dma_start(
                                k2[nch * 128:(nch + 1) * 128, w0:w0 + nw, 128:256],
                                kcur[:, cs:L].rearrange("p (w i) -> p w i", i=128))
                        # backward-key phase for the NEXT window
                        if s + L <= HALF:             # last chunk (w=15) has no next
                            kprv = rop.tile([128, 512], F32R, tag="ro")
                            _rope(nc, tmpp, craw, kprv, L, rp, 4, 5)
                            w0 = s // 128
                            nw = L // 128
                            nc.sync.dma_start(
                                k2[nch * 128:(nch + 1) * 128, w0:w0 + nw, 0:128],
                                kprv[:, 0:L].rearrange("p (w i) -> p w i", i=128))

                    # v in natural layout
                    for mt in range(nmt):
                        vsb = vp.tile([128, D], F32R, tag="v")
                        for nh in range(2):
                            vq = mps.tile([128, 512], F32, tag="mm")
                            for kc in range(8):
                                nc.tensor.matmul(vq[:],
                                                 xT[:, kc, mt * 128:(mt + 1) * 128],
                                                 w_sb[:, kc, 2048 + nh * 512: 2048 + (nh + 1) * 512],
                                                 start=(kc == 0), stop=(kc == 7))
                            nc.vector.tensor_copy(vsb[:, nh * 512:(nh + 1) * 512], vq[:])
                        nc.sync.dma_start(vstage[s + mt * 128: s + (mt + 1) * 128, :], vsb[:])

            # ---------------- Phase C: windowed attention ----------------
            with ExitStack() as cd:
                atp = cd.enter_context(tc.tile_pool(name="aT", bufs=1))
                aT = atp.tile([128, 8, HALF], F32R, tag="aT")
                with ExitStack() as cc:
                    qwp = cc.enter_context(tc.tile_pool(name="qw", bufs=2))
                    k2p = cc.enter_context(tc.tile_pool(name="k2w", bufs=2))
                    vwp = cc.enter_context(tc.tile_pool(name="vw", bufs=4))
                    ep = cc.enter_context(tc.tile_pool(name="e", bufs=3))
                    pp = cc.enter_context(tc.tile_pool(name="p", bufs=3))
                    ptp = cc.enter_context(tc.tile_pool(name="pt", bufs=3))
                    sump = cc.enter_context(tc.tile_pool(name="sums", bufs=4))
                    sps = cc.enter_context(tc.tile_pool(name="sps", bufs=3, space="PSUM"))
                    tps2 = cc.enter_context(tc.tile_pool(name="tps2", bufs=2, space="PSUM"))
                    aps = cc.enter_context(tc.tile_pool(name="aps", bufs=3, space="PSUM"))

                    vtiles = {}
                    for w in range(NWIN):
                        qw = qwp.tile([128, 8, 128], F32R, tag="qw")
                        nc.sync.dma_start(
                            qw[:], qrope[:, w * 128:(w + 1) * 128]
                            .rearrange("(c p) m -> p c m", p=128))
                        k2w = k2p.tile([128, 8, 256], F32R, tag="k2w")
                        nc.sync.dma_start(
                            k2w[:], k2[:, w, :].rearrange("(c p) j -> p c j", p=128))
                        for vt in ([w, w + 1] if w == 0 else [w + 1]):
                            v_t = vwp.tile([128, D], F32R, tag="vw")
                            nc.sync.dma_start(v_t[:], vstage[vt * 128:(vt + 1) * 128, :])
                            vtiles[vt] = v_t
                        mslot = 0 if w == 0 else 1
                        for h in range(H):
                            blk, po = h // 2, (h % 2) * 64
                            sp = sps.tile([128, 256], F32, tag="s")
                            nc.tensor.matmul(sp[:], qw[po:po + 64, blk, :],
                                             k2w[po:po + 64, blk, :],
                                             start=True, stop=True)
                            em = ep.tile([128, 256], F32, tag="em")
                            nc.vector.tensor_tensor(em[:], sp[:], mk[:, mslot, :], ADD)
                            ee = ep.tile([128, 256], F32, tag="ee")
                            ssum = sump.tile([128, 1], F32, tag="ss")
                            nc.scalar.activation(ee[:], em[:], EXP, accum_out=ssum[:])
                            rr = sump.tile([128, 1], F32, tag="rr")
                            nc.vector.reciprocal(rr[:], ssum[:])
                            pf = pp.tile([128, 256], F32R, tag="pf")
                            nc.vector.tensor_scalar_mul(pf[:], ee[:], rr[:])
                            ptq = tps2.tile([128, 256], F32R, tag="ptq")
                            nc.tensor.transpose(ptq[:, 0:128], pf[:, 0:128], ident[:])
                            nc.tensor.transpose(ptq[:, 128:256], pf[:, 128:256], ident[:])
                            pt = ptp.tile([128, 256], F32R, tag="pt")
                            nc.scalar.copy(pt[:], ptq[:])
                            ap_ = aps.tile([64, 128], F32, tag="ap")
                            nc.tensor.matmul(ap_[:], vtiles[w][:, h * 64:(h + 1) * 64],
                                             pt[:, 0:128], start=True, stop=False)
                            nc.tensor.matmul(ap_[:], vtiles[w + 1][:, h * 64:(h + 1) * 64],
                                             pt[:, 128:256], start=False, stop=True)
                            nc.scalar.copy(aT[po:po + 64, blk, w * 128:(w + 1) * 128], ap_[:])
                        vtiles.pop(w - 1, None)

                # ---------------- Phase D: output projection ----------------
                with ExitStack() as dd:
                    wop = dd.enter_context(tc.tile_pool(name="wo", bufs=1))
                    wo = wop.tile([128, 8, D], F32R, tag="wo")
                    nc.sync.dma_start(wo[:], wout.rearrange("(c p) n -> p c n", p=128))
                    outp = dd.enter_context(tc.tile_pool(name="outsb", bufs=3))
                    ops = dd.enter_context(tc.tile_pool(name="ops", bufs=4, space="PSUM"))
                    for mt in range(16):
                        osb = outp.tile([128, D], F32, tag="o")
                        for nh in range(2):
                            op_ = ops.tile([128, 512], F32, tag="op")
                            for kc in range(8):
                                nc.tensor.matmul(op_[:],
                                                 aT[:, kc, mt * 128:(mt + 1) * 128],
                                                 wo[:, kc, nh * 512:(nh + 1) * 512],
                                                 start=(kc == 0), stop=(kc == 7))
                            nc.vector.tensor_copy(osb[:, nh * 512:(nh + 1) * 512], op_[:])
                        nc.sync.dma_start(out[mt * 128:(mt + 1) * 128, :], osb[:])

    nc.compile()
    return nc


_NC = None


def _get_nc():
    global _NC
    if _NC is None:
        _NC = _build()
    return _NC


def _host_inputs(x, W_qkv, W_out):
    d = np.arange(64)
    invf = THETA ** (-(np.arange(0, 64, 2) / 64.0))          # [32]
    fr = invf[d % 32]                                        # [64]
    # sin tiles are indexed by SOURCE row of the rotate_half swap (see _rope):
    # value at row r is sign(r^32)*sin = -sign(r)*sin, hence +1 for d<32.
    sign = np.where(d < 32, 1.0, -1.0)                       # [64]
    rows_f = np.tile(fr, 2)                                  # [128] 2 head-blocks
    rows_s = np.tile(sign, 2)
    mcol = np.arange(512) % 128
    angC = rows_f[:, None] * (128 + mcol)[None, :]
    angP = rows_f[:, None] * mcol[None, :]
    ropes = np.stack([
        SCALE * np.cos(angC),
        SCALE * (rows_s[:, None] * np.sin(angC)),
        np.cos(angC),
        rows_s[:, None] * np.sin(angC),
        np.cos(angP),
        rows_s[:, None] * np.sin(angP),
    ]).astype(np.float32)

    i = np.arange(128)[:, None]
    jj = np.arange(256)[None, :]
    band = (jj >= i) & (jj <= i + 128)
    maskB = np.where(band, 0.0, NEG).astype(np.float32)
    maskA0 = np.where(band & (jj >= 128), 0.0, NEG).astype(np.float32)

    in_maps = []
    for c in range(NCORES):
        bi, hi = c // 2, c % 2
        xsh = np.empty((NT, D), np.float32)
        if hi == 0:
            xsh[:WS] = 0.0
            xsh[WS:] = x[bi, 0:HALF]
            mA = maskA0
        else:
            xsh[:] = x[bi, HALF - WS: N]
            mA = maskB
        in_maps.append({
            "xs": xsh,
            "wqkv": np.ascontiguousarray(W_qkv, np.float32),
            "wout": np.ascontiguousarray(W_out, np.float32),
            "ropes": ropes,
            "masks": np.stack([mA, maskB]),
        })
    return in_maps


def kernel(x, W_qkv, W_out):
    x = np.asarray(x, np.float32)
    nc = _get_nc()
    in_maps = _host_inputs(x, W_qkv, W_out)
    res = run_bass_kernel_spmd(nc, in_maps, list(range(NCORES)))
    outf = np.empty((B, N, D), np.float32)
    for c in range(NCORES):
        bi, hi = c // 2, c % 2
        outf[bi, hi * HALF:(hi + 1) * HALF] = res.results[c]["out"]
    return outf


# revision 6
# speedup vs baseline: 54.3211x; 54.3211x over previous
"""LocalMHA (windowed attention, window=128, look_backward=1, RoPE) on 8 TRN2 cores.

Sharding: sequence-parallel. Core c handles batch c//2, sequence half c%2
(2048 query tokens + a 128-token look-backward halo). No collectives: the
halo tokens are re-projected locally (halo rows of x are shipped in the
core's input shard; zeros at a true sequence start, where the mask kills
the backward keys anyway).

Per-core pipeline (all matmuls in fp32r — full-rate on PE, ~1.6e-4 rel err):
  A. x tiles -> PE transpose -> xT [dim, tok] chunks in SBUF
  B. QKV projection. q^T and k^T produced head-transposed ([h*dh, tok]);
     RoPE applied on the fly (k twice: once with its own-window phase, once
     with next-window backward phase) and staged to DRAM. v produced in
     natural [tok, h*dh] layout and staged.
  C. Per window w, per head: scores = q'^T.T @ k2 (free dim 256), banded
     causal mask add (DVE), exp+rowsum (ACT, fused accum), normalize (DVE),
     PE-transpose p, attn_out^T = v.T-chunks @ p^T accumulated; written to a
     resident SBUF accumulator aT [h*dh, 2048].
  D. out = aT.T @ W_out, streamed to DRAM.
"""

import numpy as np
from contextlib import ExitStack

import concourse.bacc as bacc
import concourse.tile as tile
import concourse.mybir as mybir
from concourse.bass_utils import run_bass_kernel_spmd
from concourse.masks import make_identity

# Problem shape (hardcoded per contract)
B, N, D = 4, 4096, 1024
H, DH, WS = 16, 64, 128
THETA = 10000.0
N3 = 3 * H * DH            # 3072
NCORES = 8
HALF = N // 2              # 2048 query tokens per core
NT = HALF + WS             # 2176 tokens incl halo window
NWIN = HALF // WS          # 16 query windows
SCALE = DH ** -0.5
NEG = -1.0e9

F32 = mybir.dt.float32
F32R = mybir.dt.float32r
ADD = mybir.AluOpType.add
MUL = mybir.AluOpType.mult
EXP = mybir.ActivationFunctionType.Exp

# token chunks for phases A/B (start, len); 128-aligned, len<=512
CHUNKS = [(0, 512), (512, 512), (1024, 512), (1536, 512), (2048, 128)]


def _rope(nc, tmpp, craw, dst, L, rp, ci, si):
    """dst[:, :L] = craw*cos + swap64(craw)*sin_signed (f32r out).

    craw rows are two 64-row head blocks; rotate_half exchanges the 32-row
    halves within each block. The sin tile is shipped indexed by SOURCE row
    (sign flipped host-side: sin(theta_r) is invariant under r^32, only the
    sign changes), so both tensor_tensor inputs share a base partition —
    only the output is partition-shifted, which the ISA allows.
    """
    t1 = tmpp.tile([128, 512], F32, tag="t1")
    nc.vector.tensor_tensor(t1[:, :L], craw[:, :L], rp[:, ci, :L], MUL)
    t2 = tmpp.tile([128, 512], F32, tag="t2")
    for hb in range(2):
        o = hb * 64
        nc.vector.tensor_tensor(t2[o:o + 32, :L], craw[o + 32:o + 64, :L],
                                rp[o + 32:o + 64, si, :L], MUL)
        nc.vector.tensor_tensor(t2[o + 32:o + 64, :L], craw[o:o + 32, :L],
                                rp[o:o + 32, si, :L], MUL)
    nc.vector.tensor_tensor(dst[:, :L], t1[:, :L], t2[:, :L], ADD)


def _build(reps=1):
    nc = bacc.Bacc("TRN2", target_bir_lowering=False, debug=False,
                   enable_asserts=False, num_devices=NCORES)

    xs = nc.dram_tensor("xs", [NT, D], F32R, kind="ExternalInput").ap()
    wqkv = nc.dram_tensor("wqkv", [D, N3], F32R, kind="ExternalInput").ap()
    wout = nc.dram_tensor("wout", [D, D], F32R, kind="ExternalInput").ap()
    # 0:qcos 1:qsin 2:kcos_cur 3:ksin_cur 4:kcos_prev 5:ksin_prev
    ropes = nc.dram_tensor("ropes", [6, 128, 512], F32, kind="ExternalInput").ap()
    masks = nc.dram_tensor("masks", [2, 128, 256], F32, kind="ExternalInput").ap()
    out = nc.dram_tensor("out", [HALF, D], F32, kind="ExternalOutput").ap()

    # internal DRAM staging
    qrope = nc.dram_tensor("qrope", [D, HALF], F32R).ap()
    k2 = nc.dram_tensor("k2", [D, NWIN, 2 * WS], F32R).ap()
    vstage = nc.dram_tensor("vstage", [NT, D], F32R).ap()

    with tile.TileContext(nc) as tc:
        with ExitStack() as top:
            constp = top.enter_context(tc.tile_pool(name="const", bufs=1))
            identf = constp.tile([128, 128], F32, tag="idf")
            make_identity(nc, identf[:])
            ident = constp.tile([128, 128], F32R, tag="idr")
            nc.vector.tensor_copy(ident[:], identf[:])
            rp = constp.tile([128, 6, 512], F32, tag="ropes")
            nc.sync.dma_start(rp[:], ropes.rearrange("r p m -> p r m"))
            mk = constp.tile([128, 2, 256], F32, tag="masks")
            nc.sync.dma_start(mk[:], masks.rearrange("r p m -> p r m"))

            rep_ctx = tc.For_i(0, reps, 1) if reps > 1 else ExitStack()
            top.enter_context(rep_ctx)

            # ---------------- Phase A+B: transpose + QKV + RoPE ----------------
            with ExitStack() as ab:
                wp = ab.enter_context(tc.tile_pool(name="wq", bufs=1))
                w_sb = wp.tile([128, 8, N3], F32R, tag="w")
                nc.sync.dma_start(w_sb[:],
                                  wqkv.rearrange("(c p) n -> p c n", p=128))
                xp = ab.enter_context(tc.tile_pool(name="xst", bufs=2))
                xtp = ab.enter_context(tc.tile_pool(name="xT", bufs=2))
                crp = ab.enter_context(tc.tile_pool(name="craw", bufs=3))
                tmpp = ab.enter_context(tc.tile_pool(name="tmp", bufs=2))
                rop = ab.enter_context(tc.tile_pool(name="ro", bufs=4))
                vp = ab.enter_context(tc.tile_pool(name="vsb", bufs=3))
                tps = ab.enter_context(tc.tile_pool(name="tps", bufs=3, space="PSUM"))
                mps = ab.enter_context(tc.tile_pool(name="mps", bufs=4, space="PSUM"))

                for (s, L) in CHUNKS:
                    nmt = L // 128
                    xT = xtp.tile([128, 8, 512], F32R, tag="xT")
                    for mt in range(nmt):
                        xst = xp.tile([128, D], F32R, tag="x")
                        nc.sync.dma_start(xst[:], xs[s + mt * 128: s + (mt + 1) * 128, :])
                        for kc in range(8):
                            tp = tps.tile([128, 128], F32R, tag="tp")
                            nc.tensor.transpose(tp[:], xst[:, kc * 128:(kc + 1) * 128],
                                                ident[:])
                            nc.vector.tensor_copy(xT[:, kc, mt * 128:(mt + 1) * 128], tp[:])

                    # q^T (+rope) for query tokens of this chunk
                    qs = 128 if s == 0 else 0
                    qL = L - qs
                    if qL > 0:
                        for nch in range(8):
                            qp = mps.tile([128, 512], F32, tag="mm")
                            for kc in range(8):
                                nc.tensor.matmul(qp[:, :qL],
                                                 w_sb[:, kc, nch * 128:(nch + 1) * 128],
                                                 xT[:, kc, qs:qs + qL],
                                                 start=(kc == 0), stop=(kc == 7))
                            craw = crp.tile([128, 512], F32, tag="craw")
                            nc.scalar.copy(craw[:, :qL], qp[:, :qL])
                            qf = rop.tile([128, 512], F32R, tag="ro")
                            _rope(nc, tmpp, craw, qf, qL, rp, 0, 1)
                            q0 = s + qs - 128
                            nc.sync.dma_start(
                                qrope[nch * 128:(nch + 1) * 128, q0:q0 + qL],
                                qf[:, :qL])

                    # k^T with both rope phases
                    for nch in range(8):
                        kp = mps.tile([128, 512], F32, tag="mm")
                        for kc in range(8):
                            nc.tensor.matmul(kp[:, :L],
                                             w_sb[:, kc, 1024 + nch * 128: 1024 + (nch + 1) * 128],
                                             xT[:, kc, 0:L],
                                             start=(kc == 0), stop=(kc == 7))
                        craw = crp.tile([128, 512], F32, tag="craw")
                        nc.scalar.copy(craw[:, :L], kp[:, :L])
                        # current-window phase: token t -> K2[w(t)-1? no: w_cur, 128:256]
                        cs = 128 if s == 0 else 0     # halo window has no cur slot
                        if L - cs > 0:
                            kcur = rop.tile([128, 512], F32R, tag="ro")
                            _rope(nc, tmpp, craw, kcur, L, rp, 2, 3)
                            w0 = (s + cs) // 128 - 1
                            nw = (L - cs) // 128
                            nc.sync.dma_start(
                                k2[nch * 128:(nch + 1) * 128, w0:w0 + nw, 128:256],
                                kcur[:, cs:L].rearrange("p (w i) -> p w i", i=128))
                        # backward-key phase for the NEXT window
                        if s + L <= HALF:             # last chunk (w=15) has no next
                            kprv = rop.tile([128, 512], F32R, tag="ro")
                            _rope(nc, tmpp, craw, kprv, L, rp, 4, 5)
                            w0 = s // 128
                            nw = L // 128
                            nc.sync.dma_start(
                                k2[nch * 128:(nch + 1) * 128, w0:w0 + nw, 0:128],
                                kprv[:, 0:L].rearrange("p (w i) -> p w i", i=128))

                    # v in natural layout
                    for mt in range(nmt):
                        vsb = vp.tile([128, D], F32R, tag="v")
                        for nh in range(2):
                            vq = mps.tile([128, 512], F32, tag="mm")
                            for kc in range(8):
                                nc.tensor.matmul(vq[:],
                                                 xT[:, kc, mt * 128:(mt + 1) * 128],
                                                 w_sb[:, kc, 2048 + nh * 512: 2048 + (nh + 1) * 512],
                                                 start=(kc == 0), stop=(kc == 7))
                            nc.vector.tensor_copy(vsb[:, nh * 512:(nh + 1) * 512], vq[:])
                        nc.sync.dma_start(vstage[s + mt * 128: s + (mt + 1) * 128, :], vsb[:])

            # ---------------- Phase C: windowed attention ----------------
            with ExitStack() as cd:
                atp = cd.enter_context(tc.tile_pool(name="aT", bufs=1))
                aT = atp.tile([128, 8, HALF], F32R, tag="aT")
                with ExitStack() as cc:
                    qwp = cc.enter_context(tc.tile_pool(name="qw", bufs=2))
                    k2p = cc.enter_context(tc.tile_pool(name="k2w", bufs=2))
                    vwp = cc.enter_context(tc.tile_pool(name="vw", bufs=4))
                    ep = cc.enter_context(tc.tile_pool(name="e", bufs=3))
                    pp = cc.enter_context(tc.tile_pool(name="p", bufs=3))
                    ptp = cc.enter_context(tc.tile_pool(name="pt", bufs=3))
                    sump = cc.enter_context(tc.tile_pool(name="sums", bufs=4))
                    sps = cc.enter_context(tc.tile_pool(name="sps", bufs=3, space="PSUM"))
                    tps2 = cc.enter_context(tc.tile_pool(name="tps2", bufs=2, space="PSUM"))
                    aps = cc.enter_context(tc.tile_pool(name="aps", bufs=3, space="PSUM"))

                    vtiles = {}
                    for w in range(NWIN):
                        qw = qwp.tile([128, 8, 128], F32R, tag="qw")
                        nc.sync.dma_start(
                            qw[:], qrope[:, w * 128:(w + 1) * 128]
                            .rearrange("(c p) m -> p c m", p=128))
                        k2w = k2p.tile([128, 8, 256], F32R, tag="k2w")
                        nc.sync.dma_start(
                            k2w[:], k2[:, w, :].rearrange("(c p) j -> p c j", p=128))
                        for vt in ([w, w + 1] if w == 0 else [w + 1]):
                            v_t = vwp.tile([128, D], F32R, tag="vw")
                            nc.sync.dma_start(v_t[:], vstage[vt * 128:(vt + 1) * 128, :])
                            vtiles[vt] = v_t
                        mslot = 0 if w == 0 else 1
                        for h in range(H):
                            blk, po = h // 2, (h % 2) * 64
                            sp = sps.tile([128, 256], F32, tag="s")
                            nc.tensor.matmul(sp[:], qw[po:po + 64, blk, :],
                                             k2w[po:po + 64, blk, :],
                                             start=True, stop=True)
                            em = ep.tile([128, 256], F32, tag="em")
                            nc.vector.tensor_tensor(em[:], sp[:], mk[:, mslot, :], ADD)
                            ee = ep.tile([128, 256], F32, tag="ee")
                            ssum = sump.tile([128, 1], F32, tag="ss")
                            nc.scalar.activation(ee[:], em[:], EXP, accum_out=ssum[:])
                            rr = sump.tile([128, 1], F32, tag="rr")
                            nc.vector.reciprocal(rr[:], ssum[:])
                            pf = pp.tile([128, 256], F32R, tag="pf")
                            nc.vector.tensor_scalar_mul(pf[:], ee[:], rr[:])
                            ptq = tps2.tile([128, 256], F32R, tag="ptq")
                            nc.tensor.transpose(ptq[:, 0:128], pf[:, 0:128], ident[:])
                            nc.tensor.transpose(ptq[:, 128:256], pf[:, 128:256], ident[:])
                            pt = ptp.tile([128, 256], F32R, tag="pt")
                            nc.scalar.copy(pt[:], ptq[:])
                            ap_ = aps.tile([64, 128], F32, tag="ap")
                            nc.tensor.matmul(ap_[:], vtiles[w][:, h * 64:(h + 1) * 64],
                                             pt[:, 0:128], start=True, stop=False)
                            nc.tensor.matmul(ap_[:], vtiles[w + 1][:, h * 64:(h + 1) * 64],
                                             pt[:, 128:256], start=False, stop=True)
                            nc.scalar.copy(aT[po:po + 64, blk, w * 128:(w + 1) * 128], ap_[:])
                        vtiles.pop(w - 1, None)

                # ---------------- Phase D: output projection ----------------
                with ExitStack() as dd:
                    wop = dd.enter_context(tc.tile_pool(name="wo", bufs=1))
                    wo = wop.tile([128, 8, D], F32R, tag="wo")
                    nc.sync.dma_start(wo[:], wout.rearrange("(c p) n -> p c n", p=128))
                    outp = dd.enter_context(tc.tile_pool(name="outsb", bufs=3))
                    ops = dd.enter_context(tc.tile_pool(name="ops", bufs=4, space="PSUM"))
                    for mt in range(16):
                        osb = outp.tile([128, D], F32, tag="o")
                        for nh in range(2):
                            op_ = ops.tile([128, 512], F32, tag="op")
                            for kc in range(8):
                                nc.tensor.matmul(op_[:],
                                                 aT[:, kc, mt * 128:(mt + 1) * 128],
                                                 wo[:, kc, nh * 512:(nh + 1) * 512],
                                                 start=(kc == 0), stop=(kc == 7))
                            nc.vector.tensor_copy(osb[:, nh * 512:(nh + 1) * 512], op_[:])
                        nc.sync.dma_start(out[mt * 128:(mt + 1) * 128, :], osb[:])

    nc.compile()
    return nc


_NC = {}


def _get_nc(reps=1):
    if reps not in _NC:
        _NC[reps] = _build(reps)
    return _NC[reps]


def _host_inputs(x, W_qkv, W_out):
    d = np.arange(64)
    invf = THETA ** (-(np.arange(0, 64, 2) / 64.0))          # [32]
    fr = invf[d % 32]                                        # [64]
    # sin tiles are indexed by SOURCE row of the rotate_half swap (see _rope):
    # value at row r is sign(r^32)*sin = -sign(r)*sin, hence +1 for d<32.
    sign = np.where(d < 32, 1.0, -1.0)                       # [64]
    rows_f = np.tile(fr, 2)                                  # [128] 2 head-blocks
    rows_s = np.tile(sign, 2)
    mcol = np.arange(512) % 128
    angC = rows_f[:, None] * (128 + mcol)[None, :]
    angP = rows_f[:, None] * mcol[None, :]
    ropes = np.stack([
        SCALE * np.cos(angC),
        SCALE * (rows_s[:, None] * np.sin(angC)),
        np.cos(angC),
        rows_s[:, None] * np.sin(angC),
        np.cos(angP),
        rows_s[:, None] * np.sin(angP),
    ]).astype(np.float32)

    i = np.arange(128)[:, None]
    jj = np.arange(256)[None, :]
    band = (jj >= i) & (jj <= i + 128)
    maskB = np.where(band, 0.0, NEG).astype(np.float32)
    maskA0 = np.where(band & (jj >= 128), 0.0, NEG).astype(np.float32)

    in_maps = []
    for c in range(NCORES):
        bi, hi = c // 2, c % 2
        xsh = np.empty((NT, D), np.float32)
        if hi == 0:
            xsh[:WS] = 0.0
            xsh[WS:] = x[bi, 0:HALF]
            mA = maskA0
        else:
            xsh[:] = x[bi, HALF - WS: N]
            mA = maskB
        in_maps.append({
            "xs": xsh,
            "wqkv": np.ascontiguousarray(W_qkv, np.float32),
            "wout": np.ascontiguousarray(W_out, np.float32),
            "ropes": ropes,
            "masks": np.stack([mA, maskB]),
        })
    return in_maps


def kernel(x, W_qkv, W_out):
    x = np.asarray(x, np.float32)
    nc = _get_nc()
    in_maps = _host_inputs(x, W_qkv, W_out)
    res = run_bass_kernel_spmd(nc, in_maps, list(range(NCORES)))
    outf = np.empty((B, N, D), np.float32)
    for c in range(NCORES):
        bi, hi = c // 2, c % 2
        outf[bi, hi * HALF:(hi + 1) * HALF] = res.results[c]["out"]
    return outf
